# revision 32
# baseline (speedup 1.0000x reference)
"""BitLinear (int4-fakequant x @ ternary-weight linear) Trainium2 Bass kernel.

Math (per reference):
  maxabs[s] = max(|x[s, :]|) clamped to >= 1e-6
  q[s, k]   = round(x[s, k] / maxabs[s] * 7)           # in [-7, 7]
  xq        = q * maxabs / 7
  thresh    = 0.05 * mean(|w|)                          # global scalar
  sign[o,k] = 0 if |w[o,k]| < thresh else sign(w[o,k])  # in {-1, 0, 1}
  alpha[o]  = mean(|w[o, :]|)
  out[s, o] = (maxabs[s]/7) * alpha[o] * S[s,o] + bias[o],  S = q @ sign.T

S is an exact small-integer matmul computed on the PE array in fp8 e4m3 with
DoubleRow (ints -8..7 exact; fp32 accumulation, |S| <= 28672 < 2^24 -> exact).
Per-partition row scale (maxabs/7) applied on ACT during PSUM eviction;
per-column alpha applied on DVE in bf16 (2x mode); output shipped bf16.

Sharding: column-parallel over out_f across 8 cores (weight/alpha/out
sharded, x replicated). Host does layout/stat prep (transposes, row stats,
pre-scale x by 7/maxabs, ternarize w -> fp8 sign), mirroring the baseline's
host-side stats; all O(N*K*O) matmul work plus the quantization rounding
nonlinearity runs on device.

HW-measured facts this schedule is built on (from the baseline trace):
  - a DoubleRow fp8 matmul with N output columns takes ~N/2.4GHz + 3ns; the
    fp8-DR wall is 1 out-col/cycle at 2.4 GHz (157 TF/s).
  - LDWEIGHTS (135 ns) fully hides under the 216 ns matmuls.
  => per-core floor = 64 blocks * 64 MMs * ~216 ns ~= 884 us. The previous
     kernel spent ~1113 us: ~65 us weight-prep serial phase at start plus
     ~150 us of PE gaps from DVE (inv7 multiply) and DMA contention.
"""

import numpy as np
import ml_dtypes

import concourse.bacc as bacc
import concourse.bass as bass
import concourse.mybir as mybir
import concourse.tile as tile
from concourse.bass import ts

F32 = mybir.dt.float32
BF16 = mybir.dt.bfloat16
FP8 = mybir.dt.float8e4
AOP = mybir.AluOpType
ACTF = mybir.ActivationFunctionType

P = 128
OTILE = 512          # psum free-dim tile (one bank of fp32)
MCHUNK = 256         # m-columns per x DMA/quant chunk (2 MM-blocks)
# adding/subtracting this forces RNE round-to-integer in fp32
MAGIC = 1.5 * 2.0 ** 23


def build_nc(M, IN_F, O_SH, with_bias):
    """Per-core SPMD program. Shapes are per-core shard shapes."""
    KSUB = IN_F // P          # k-subtiles (pairs for DoubleRow)
    NPAIR = KSUB // 2
    NCH = M // MCHUNK         # x chunks
    BPC = MCHUNK // P         # MM-blocks per chunk
    NBLK = M // P
    NOT = O_SH // OTILE
    assert KSUB % 2 == 0 and M % MCHUNK == 0 and O_SH % OTILE == 0

    nc = bacc.Bacc("TRN2", target_bir_lowering=False, debug=False)

    # All bulk inputs are packed per-partition-contiguous on the host so each
    # DMA issues one 16-32KB descriptor per partition (512B-line layouts
    # measured only ~130 GB/s).
    # xs: pre-scaled x^T chunks, [NCH, P, KSUB, MCHUNK]
    xs = nc.dram_tensor("xs", [NCH, P, KSUB, MCHUNK], F32, kind="ExternalInput").ap()
    # xs01: chunks 0-1 duplicated as 128-m halves for the fast start
    xs01 = nc.dram_tensor("xs01", [2, BPC, P, KSUB, P], F32, kind="ExternalInput").ap()
    # sg: ternary sign o-quarters, [NOT, P, KSUB, OTILE] fp8
    sg = nc.dram_tensor("sg", [NOT, P, KSUB, OTILE], FP8, kind="ExternalInput").ap()
    rs = nc.dram_tensor("rs", [P, NBLK], F32, kind="ExternalInput").ap()
    alpha = nc.dram_tensor("alpha", [1, O_SH], BF16, kind="ExternalInput").ap()
    if with_bias:
        bias = nc.dram_tensor("bias", [1, O_SH], F32, kind="ExternalInput").ap()
    out = nc.dram_tensor("out", [M, O_SH], BF16, kind="ExternalOutput").ap()

    out_r = out.rearrange("(t p) o -> p t o", p=P)       # [128, NBLK, O_SH]

    with tile.TileContext(nc) as tc:
        with (
            tc.tile_pool(name="const", bufs=1) as constp,
            tc.tile_pool(name="sign", bufs=1) as signp,
            tc.tile_pool(name="xin", bufs=2) as xin,
            tc.tile_pool(name="mini", bufs=1) as minip,
            tc.tile_pool(name="q8p", bufs=3) as q8p,
            tc.tile_pool(name="outp", bufs=2) as outp,
            tc.tile_pool(name="psum", bufs=8, space="PSUM") as psum,
        ):
            # ---- constants (tiny, ahead of sign on the ACT DMA ring) ----
            rs_sb = constp.tile([P, NBLK], F32, tag="rs_sb")
            nc.scalar.dma_start(rs_sb[:], rs[:, :])
            alpha_bc = constp.tile([P, O_SH], BF16, tag="alpha_bc")
            nc.scalar.dma_start(alpha_bc[:], alpha[0:1, :].to_broadcast((P, O_SH)))
            if with_bias:
                bias_bc = constp.tile([P, O_SH], F32, tag="bias_bc")
                nc.scalar.dma_start(bias_bc[:], bias[0:1, :].to_broadcast((P, O_SH)))

            # ---- sign tiles: straight fp8 DMA, one per o-quarter; quarter q
            # is first consumed ~7*q us into the run ----
            # one tile per quarter (per-tile dep tracking: a matmul on
            # quarter q must not wait for the other quarters' DMAs), loads
            # split across both DMA rings so arrival tracks consumption:
            # q0/q1 on the sync ring interleaved with the first x minis,
            # q2/q3 on the scalar ring behind rs/alpha
            sign_ts = [
                signp.tile([P, KSUB, OTILE], FP8, tag=f"sign{q}", name=f"sign{q}")
                for q in range(NOT)
            ]
            nc.sync.dma_start(sign_ts[0][:], sg[0])
            for q in range(2, NOT):
                nc.scalar.dma_start(sign_ts[q][:], sg[q])

            def quant_chunk(c):
                xt_t = xin.tile([P, KSUB, MCHUNK], F32, tag="xt", name=f"xt_{c}")
                q8_t = q8p.tile([P, KSUB, MCHUNK], FP8, tag="q8", name=f"q8_{c}")
                nc.sync.dma_start(xt_t[:], xs[c])
                nc.vector.tensor_scalar(
                    q8_t[:], xt_t[:], MAGIC, -MAGIC, AOP.add, AOP.add
                )
                return q8_t

            def quant_minis(c, sign_after=None):
                # chunk as two independent 128-m tiles: block (c,0) unblocks
                # after 2 MB of x instead of 4 (tile-granular dep tracking)
                outs = []
                for h in range(BPC):
                    xt_t = minip.tile(
                        [P, KSUB, P], F32, tag=f"xtm{h}", name=f"xtm_{c}_{h}"
                    )
                    nc.sync.dma_start(xt_t[:], xs01[c, h])
                    if sign_after is not None and h == 0:
                        # q1 rides the sync ring between the two minis
                        nc.sync.dma_start(sign_ts[sign_after][:], sg[sign_after])
                    q8_t = minip.tile(
                        [P, KSUB, P], FP8, tag=f"q8m{h}", name=f"q8m_{c}_{h}"
                    )
                    nc.vector.tensor_scalar(
                        q8_t[:], xt_t[:], MAGIC, -MAGIC, AOP.add, AOP.add
                    )
                    outs.append(q8_t)
                return outs

            # ---- main pipeline: supergroup == one m-chunk (2 blocks); the
            # o-quarter loop is OUTSIDE the block loop so quarter q of sign
            # is needed only q*2*3.45us into each supergroup — the PE rides
            # the sign DMA arrival curve instead of waiting for all of it
            q8_next = [None] * (NCH + 1)
            q8_next[0] = quant_minis(0, sign_after=1)
            for c in range(NCH):
                q8_t = q8_next[c]
                minis = isinstance(q8_t, list)
                if c + 2 < NCH:
                    q8_next[c + 2] = quant_chunk(c + 2)
                obs = [
                    outp.tile([P, O_SH], BF16, tag="ob", name=f"ob_{c}_{b}")
                    for b in range(BPC)
                ]
                for q in range(NOT):
                    # chunk 1 queues only after q0/miniA/q1/miniB: keeps the
                    # first-supergroup DMA window free of not-yet-needed bytes
                    if c == 0 and q == 1 and NCH > 1 and q8_next[1] is None:
                        q8_next[1] = quant_chunk(1)
                    for b in range(BPC):
                        t = c * BPC + b      # global m-block id
                        lq = q8_t[b] if minis else q8_t
                        msl = ts(0 if minis else b, P)
                        ps_t = psum.tile(
                            [P, OTILE], F32, tag="ps", name=f"ps_{t}_{q}"
                        )
                        for kk in range(NPAIR):
                            nc.tensor.matmul(
                                ps_t[:],
                                lq[:, 2 * kk : 2 * kk + 2, msl],
                                sign_ts[q][:, 2 * kk : 2 * kk + 2, :],
                                start=(kk == 0),
                                stop=(kk == NPAIR - 1),
                                perf_mode=mybir.MatmulPerfMode.DoubleRow,
                            )
                        nc.scalar.activation(
                            obs[b][:, ts(q, OTILE)],
                            ps_t[:],
                            ACTF.Copy,
                            scale=rs_sb[:, t : t + 1],
                        )
                        # per-quarter alpha + store: keeps the post-matmul
                        # tail to one quarter's chain instead of two blocks'
                        nc.vector.tensor_tensor(
                            obs[b][:, ts(q, OTILE)],
                            obs[b][:, ts(q, OTILE)],
                            alpha_bc[:, ts(q, OTILE)],
                            AOP.mult,
                        )
                        if with_bias:
                            nc.vector.tensor_tensor(
                                obs[b][:, ts(q, OTILE)],
                                obs[b][:, ts(q, OTILE)],
                                bias_bc[:, ts(q, OTILE)],
                                AOP.add,
                            )
                        nc.scalar.dma_start(
                            out_r[:, t, ts(q, OTILE)], obs[b][:, ts(q, OTILE)]
                        )

    nc.compile()
    return nc


def host_prep(x, weight, bias, n_cores):
    """Host-side layout prep + row stats + ternarize. Returns per-core maps."""
    IN_F = x.shape[-1]
    OUT_F = weight.shape[0]
    M = int(np.prod(x.shape[:-1]))
    O_SH = OUT_F // n_cores
    NBLK = M // P
    NCH = M // MCHUNK

    x2 = np.ascontiguousarray(x.reshape(M, IN_F), dtype=np.float32)
    maxabs = np.maximum(np.abs(x2).max(axis=1), 1e-6).astype(np.float32)
    # exact reference order: (x / maxabs) * 7, all fp32
    xs2 = ((x2 / maxabs[:, None]).astype(np.float32) * np.float32(7.0)).astype(
        np.float32
    )
    rs = (maxabs / np.float32(7.0)).astype(np.float32)
    rs_striped = np.ascontiguousarray(rs.reshape(NBLK, P).T)  # [128, NBLK]

    KSUB = IN_F // P
    BPC = MCHUNK // P
    xsT = xs2.T  # [IN_F, M]
    # per-partition-contiguous chunks: [NCH, P, KSUB, MCHUNK]
    xs_cm = np.ascontiguousarray(
        xsT.reshape(KSUB, P, NCH, MCHUNK).transpose(2, 1, 0, 3)
    )
    # chunks 0-1 as 128-m halves: [2, BPC, P, KSUB, P]
    xs01 = np.ascontiguousarray(
        xsT[:, : 2 * MCHUNK]
        .reshape(KSUB, P, 2, BPC, P)
        .transpose(2, 3, 1, 0, 4)
    )

    w64 = weight.astype(np.float32)
    thresh = np.float32(0.05) * np.float32(np.abs(w64).mean(dtype=np.float64))
    sign_full = np.where(np.abs(w64) < thresh, 0.0, np.sign(w64)).astype(
        ml_dtypes.float8_e4m3fn
    )  # [OUT_F, IN_F]
    alpha_full = np.abs(w64).mean(axis=1, dtype=np.float32)
    NOT = O_SH // OTILE

    with_bias = bool(np.any(bias))

    in_maps = []
    for c in range(n_cores):
        o0, o1 = c * O_SH, (c + 1) * O_SH
        sgT = sign_full[o0:o1].T  # [IN_F, O_SH] fp8
        m = {
            "xs": xs_cm,
            "xs01": xs01,
            # o-quartered per-partition-contiguous: [NOT, P, KSUB, OTILE]
            "sg": np.ascontiguousarray(
                sgT.reshape(KSUB, P, NOT, OTILE).transpose(2, 1, 0, 3)
            ),
            "rs": rs_striped,
            "alpha": alpha_full[o0:o1].astype(ml_dtypes.bfloat16).reshape(1, O_SH),
        }
        if with_bias:
            m["bias"] = np.ascontiguousarray(bias[o0:o1], dtype=np.float32).reshape(
                1, O_SH
            )
        in_maps.append(m)
    return in_maps, with_bias


_NC_CACHE = {}


def _get_nc(M, IN_F, O_SH, with_bias):
    key = (M, IN_F, O_SH, with_bias)
    if key not in _NC_CACHE:
        _NC_CACHE[key] = build_nc(M, IN_F, O_SH, with_bias)
    return _NC_CACHE[key]


def kernel(x, weight, bias, _trace=False):
    from concourse.bass_utils import run_bass_kernel_spmd

    N_CORES = 8
    x = np.asarray(x)
    weight = np.asarray(weight)
    bias = np.asarray(bias)
    IN_F = x.shape[-1]
    OUT_F = weight.shape[0]
    M = int(np.prod(x.shape[:-1]))
    O_SH = OUT_F // N_CORES

    in_maps, with_bias = host_prep(x, weight, bias, N_CORES)
    nc = _get_nc(M, IN_F, O_SH, with_bias)
    res = run_bass_kernel_spmd(
        nc, in_maps, core_ids=list(range(N_CORES)), trace=_trace
    )
    parts = [
        res.results[c]["out"].astype(np.float32).reshape(*x.shape[:-1], O_SH)
        for c in range(N_CORES)
    ]
    full = np.concatenate(parts, axis=-1)
    if with_bias is False and np.any(bias):  # pragma: no cover (safety)
        full = full + bias
    if _trace:
        return full, res
    return full


# revision 34
# speedup vs baseline: 1.0068x; 1.0068x over previous
"""BitLinear (int4-fakequant x @ ternary-weight linear) Trainium2 Bass kernel.

Math (per reference):
  maxabs[s] = max(|x[s, :]|) clamped to >= 1e-6
  q[s, k]   = round(x[s, k] / maxabs[s] * 7)           # in [-7, 7]
  xq        = q * maxabs / 7
  thresh    = 0.05 * mean(|w|)                          # global scalar
  sign[o,k] = 0 if |w[o,k]| < thresh else sign(w[o,k])  # in {-1, 0, 1}
  alpha[o]  = mean(|w[o, :]|)
  out[s, o] = (maxabs[s]/7) * alpha[o] * S[s,o] + bias[o],  S = q @ sign.T

S is an exact small-integer matmul computed on the PE array in fp8 e4m3 with
DoubleRow (ints -8..7 exact; fp32 accumulation, |S| <= 28672 < 2^24 -> exact).
Per-partition row scale (maxabs/7) applied on ACT during PSUM eviction;
per-column alpha applied on DVE in bf16 (2x mode); output shipped bf16.

Sharding: column-parallel over out_f across 8 cores (weight/alpha/out
sharded, x replicated). Host does layout/stat prep (transposes, row stats,
pre-scale x by 7/maxabs, ternarize w -> fp8 sign), mirroring the baseline's
host-side stats; all O(N*K*O) matmul work plus the quantization rounding
nonlinearity runs on device.

HW-measured facts this schedule is built on (from the baseline trace):
  - a DoubleRow fp8 matmul with N output columns takes ~N/2.4GHz + 3ns; the
    fp8-DR wall is 1 out-col/cycle at 2.4 GHz (157 TF/s).
  - LDWEIGHTS (135 ns) fully hides under the 216 ns matmuls.
  => per-core floor = 64 blocks * 64 MMs * ~216 ns ~= 884 us. The previous
     kernel spent ~1113 us: ~65 us weight-prep serial phase at start plus
     ~150 us of PE gaps from DVE (inv7 multiply) and DMA contention.
"""

import numpy as np
import ml_dtypes

import concourse.bacc as bacc
import concourse.bass as bass
import concourse.mybir as mybir
import concourse.tile as tile
from concourse.bass import ts

F32 = mybir.dt.float32
BF16 = mybir.dt.bfloat16
FP8 = mybir.dt.float8e4
AOP = mybir.AluOpType
ACTF = mybir.ActivationFunctionType

P = 128
OTILE = 512          # psum free-dim tile (one bank of fp32)
MCHUNK = 256         # m-columns per x DMA/quant chunk (2 MM-blocks)
# adding/subtracting this forces RNE round-to-integer in fp32
MAGIC = 1.5 * 2.0 ** 23


def build_nc(M, IN_F, O_SH, with_bias):
    """Per-core SPMD program. Shapes are per-core shard shapes."""
    KSUB = IN_F // P          # k-subtiles (pairs for DoubleRow)
    NPAIR = KSUB // 2
    NCH = M // MCHUNK         # x chunks
    BPC = MCHUNK // P         # MM-blocks per chunk
    NBLK = M // P
    NOT = O_SH // OTILE
    assert KSUB % 2 == 0 and M % MCHUNK == 0 and O_SH % OTILE == 0

    nc = bacc.Bacc("TRN2", target_bir_lowering=False, debug=False)

    # All bulk inputs are packed per-partition-contiguous on the host so each
    # DMA issues one 16-32KB descriptor per partition (512B-line layouts
    # measured only ~130 GB/s).
    # xs: pre-scaled x^T chunks, [NCH, P, KSUB, MCHUNK]
    xs = nc.dram_tensor("xs", [NCH, P, KSUB, MCHUNK], F32, kind="ExternalInput").ap()
    # xs01: chunks 0-1 duplicated as 128-m halves for the fast start
    xs01 = nc.dram_tensor("xs01", [2, BPC, P, KSUB, P], F32, kind="ExternalInput").ap()
    # sg: ternary sign o-quarters, [NOT, P, KSUB, OTILE] fp8
    sg = nc.dram_tensor("sg", [NOT, P, KSUB, OTILE], FP8, kind="ExternalInput").ap()
    rs = nc.dram_tensor("rs", [P, NBLK], F32, kind="ExternalInput").ap()
    alpha = nc.dram_tensor("alpha", [1, O_SH], BF16, kind="ExternalInput").ap()
    if with_bias:
        bias = nc.dram_tensor("bias", [1, O_SH], F32, kind="ExternalInput").ap()
    out = nc.dram_tensor("out", [M, O_SH], BF16, kind="ExternalOutput").ap()

    out_r = out.rearrange("(t p) o -> p t o", p=P)       # [128, NBLK, O_SH]

    with tile.TileContext(nc) as tc:
        with (
            tc.tile_pool(name="const", bufs=1) as constp,
            tc.tile_pool(name="sign", bufs=1) as signp,
            tc.tile_pool(name="xin", bufs=2) as xin,
            tc.tile_pool(name="mini", bufs=1) as minip,
            tc.tile_pool(name="q8p", bufs=3) as q8p,
            tc.tile_pool(name="outp", bufs=2) as outp,
            tc.tile_pool(name="psum", bufs=8, space="PSUM") as psum,
        ):
            # ---- constants (tiny, ahead of sign on the ACT DMA ring) ----
            rs_sb = constp.tile([P, NBLK], F32, tag="rs_sb")
            nc.scalar.dma_start(rs_sb[:], rs[:, :])
            alpha_bc = constp.tile([P, O_SH], BF16, tag="alpha_bc")
            nc.scalar.dma_start(alpha_bc[:], alpha[0:1, :].to_broadcast((P, O_SH)))
            if with_bias:
                bias_bc = constp.tile([P, O_SH], F32, tag="bias_bc")
                nc.scalar.dma_start(bias_bc[:], bias[0:1, :].to_broadcast((P, O_SH)))

            # ---- sign tiles: straight fp8 DMA, one per o-quarter; quarter q
            # is first consumed ~7*q us into the run ----
            # one tile per quarter (per-tile dep tracking: a matmul on
            # quarter q must not wait for the other quarters' DMAs), loads
            # split across both DMA rings so arrival tracks consumption:
            # q0/q1 on the sync ring interleaved with the first x minis,
            # q2/q3 on the scalar ring behind rs/alpha
            sign_ts = [
                signp.tile([P, KSUB, OTILE], FP8, tag=f"sign{q}", name=f"sign{q}")
                for q in range(NOT)
            ]
            nc.sync.dma_start(sign_ts[0][:], sg[0])
            for q in range(2, NOT):
                nc.scalar.dma_start(sign_ts[q][:], sg[q])

            def quant_chunk(c):
                xt_t = xin.tile([P, KSUB, MCHUNK], F32, tag="xt", name=f"xt_{c}")
                q8_t = q8p.tile([P, KSUB, MCHUNK], FP8, tag="q8", name=f"q8_{c}")
                nc.sync.dma_start(xt_t[:], xs[c])
                nc.vector.tensor_scalar(
                    q8_t[:], xt_t[:], MAGIC, -MAGIC, AOP.add, AOP.add
                )
                return q8_t

            def quant_minis(c, sign_after=None):
                # chunk as two independent 128-m tiles: block (c,0) unblocks
                # after 2 MB of x instead of 4 (tile-granular dep tracking)
                outs = []
                for h in range(BPC):
                    xt_t = minip.tile(
                        [P, KSUB, P], F32, tag=f"xtm{h}", name=f"xtm_{c}_{h}"
                    )
                    nc.sync.dma_start(xt_t[:], xs01[c, h])
                    if sign_after is not None and h == 0:
                        # q1 rides the sync ring between the two minis
                        nc.sync.dma_start(sign_ts[sign_after][:], sg[sign_after])
                    q8_t = minip.tile(
                        [P, KSUB, P], FP8, tag=f"q8m{h}", name=f"q8m_{c}_{h}"
                    )
                    nc.vector.tensor_scalar(
                        q8_t[:], xt_t[:], MAGIC, -MAGIC, AOP.add, AOP.add
                    )
                    outs.append(q8_t)
                return outs

            # ---- main pipeline: supergroup == one m-chunk (2 blocks); the
            # o-quarter loop is OUTSIDE the block loop so quarter q of sign
            # is needed only q*2*3.45us into each supergroup — the PE rides
            # the sign DMA arrival curve instead of waiting for all of it
            q8_next = [None] * (NCH + 1)
            q8_next[0] = quant_minis(0, sign_after=1)
            if NCH > 1:
                q8_next[1] = quant_minis(1)
            for c in range(NCH):
                q8_t = q8_next[c]
                minis = isinstance(q8_t, list)
                if c + 2 < NCH:
                    q8_next[c + 2] = quant_chunk(c + 2)
                obs = [
                    outp.tile([P, O_SH], BF16, tag="ob", name=f"ob_{c}_{b}")
                    for b in range(BPC)
                ]
                for q in range(NOT):
                    for b in range(BPC):
                        t = c * BPC + b      # global m-block id
                        lq = q8_t[b] if minis else q8_t
                        msl = ts(0 if minis else b, P)
                        ps_t = psum.tile(
                            [P, OTILE], F32, tag="ps", name=f"ps_{t}_{q}"
                        )
                        for kk in range(NPAIR):
                            nc.tensor.matmul(
                                ps_t[:],
                                lq[:, 2 * kk : 2 * kk + 2, msl],
                                sign_ts[q][:, 2 * kk : 2 * kk + 2, :],
                                start=(kk == 0),
                                stop=(kk == NPAIR - 1),
                                perf_mode=mybir.MatmulPerfMode.DoubleRow,
                            )
                        nc.scalar.activation(
                            obs[b][:, ts(q, OTILE)],
                            ps_t[:],
                            ACTF.Copy,
                            scale=rs_sb[:, t : t + 1],
                        )
                        # per-quarter alpha + store: keeps the post-matmul
                        # tail to one quarter's chain instead of two blocks'
                        nc.vector.tensor_tensor(
                            obs[b][:, ts(q, OTILE)],
                            obs[b][:, ts(q, OTILE)],
                            alpha_bc[:, ts(q, OTILE)],
                            AOP.mult,
                        )
                        if with_bias:
                            nc.vector.tensor_tensor(
                                obs[b][:, ts(q, OTILE)],
                                obs[b][:, ts(q, OTILE)],
                                bias_bc[:, ts(q, OTILE)],
                                AOP.add,
                            )
                        nc.scalar.dma_start(
                            out_r[:, t, ts(q, OTILE)], obs[b][:, ts(q, OTILE)]
                        )

    nc.compile()
    return nc


def host_prep(x, weight, bias, n_cores):
    """Host-side layout prep + row stats + ternarize. Returns per-core maps."""
    IN_F = x.shape[-1]
    OUT_F = weight.shape[0]
    M = int(np.prod(x.shape[:-1]))
    O_SH = OUT_F // n_cores
    NBLK = M // P
    NCH = M // MCHUNK

    x2 = np.ascontiguousarray(x.reshape(M, IN_F), dtype=np.float32)
    maxabs = np.maximum(np.abs(x2).max(axis=1), 1e-6).astype(np.float32)
    # exact reference order: (x / maxabs) * 7, all fp32
    xs2 = ((x2 / maxabs[:, None]).astype(np.float32) * np.float32(7.0)).astype(
        np.float32
    )
    rs = (maxabs / np.float32(7.0)).astype(np.float32)
    rs_striped = np.ascontiguousarray(rs.reshape(NBLK, P).T)  # [128, NBLK]

    KSUB = IN_F // P
    BPC = MCHUNK // P
    xsT = xs2.T  # [IN_F, M]
    # per-partition-contiguous chunks: [NCH, P, KSUB, MCHUNK]
    xs_cm = np.ascontiguousarray(
        xsT.reshape(KSUB, P, NCH, MCHUNK).transpose(2, 1, 0, 3)
    )
    # chunks 0-1 as 128-m halves: [2, BPC, P, KSUB, P]
    xs01 = np.ascontiguousarray(
        xsT[:, : 2 * MCHUNK]
        .reshape(KSUB, P, 2, BPC, P)
        .transpose(2, 3, 1, 0, 4)
    )

    w64 = weight.astype(np.float32)
    thresh = np.float32(0.05) * np.float32(np.abs(w64).mean(dtype=np.float64))
    sign_full = np.where(np.abs(w64) < thresh, 0.0, np.sign(w64)).astype(
        ml_dtypes.float8_e4m3fn
    )  # [OUT_F, IN_F]
    alpha_full = np.abs(w64).mean(axis=1, dtype=np.float32)
    NOT = O_SH // OTILE

    with_bias = bool(np.any(bias))

    in_maps = []
    for c in range(n_cores):
        o0, o1 = c * O_SH, (c + 1) * O_SH
        sgT = sign_full[o0:o1].T  # [IN_F, O_SH] fp8
        m = {
            "xs": xs_cm,
            "xs01": xs01,
            # o-quartered per-partition-contiguous: [NOT, P, KSUB, OTILE]
            "sg": np.ascontiguousarray(
                sgT.reshape(KSUB, P, NOT, OTILE).transpose(2, 1, 0, 3)
            ),
            "rs": rs_striped,
            "alpha": alpha_full[o0:o1].astype(ml_dtypes.bfloat16).reshape(1, O_SH),
        }
        if with_bias:
            m["bias"] = np.ascontiguousarray(bias[o0:o1], dtype=np.float32).reshape(
                1, O_SH
            )
        in_maps.append(m)
    return in_maps, with_bias


_NC_CACHE = {}


def _get_nc(M, IN_F, O_SH, with_bias):
    key = (M, IN_F, O_SH, with_bias)
    if key not in _NC_CACHE:
        _NC_CACHE[key] = build_nc(M, IN_F, O_SH, with_bias)
    return _NC_CACHE[key]


def kernel(x, weight, bias, _trace=False):
    from concourse.bass_utils import run_bass_kernel_spmd

    N_CORES = 8
    x = np.asarray(x)
    weight = np.asarray(weight)
    bias = np.asarray(bias)
    IN_F = x.shape[-1]
    OUT_F = weight.shape[0]
    M = int(np.prod(x.shape[:-1]))
    O_SH = OUT_F // N_CORES

    in_maps, with_bias = host_prep(x, weight, bias, N_CORES)
    nc = _get_nc(M, IN_F, O_SH, with_bias)
    res = run_bass_kernel_spmd(
        nc, in_maps, core_ids=list(range(N_CORES)), trace=_trace
    )
    parts = [
        res.results[c]["out"].astype(np.float32).reshape(*x.shape[:-1], O_SH)
        for c in range(N_CORES)
    ]
    full = np.concatenate(parts, axis=-1)
    if with_bias is False and np.any(bias):  # pragma: no cover (safety)
        full = full + bias
    if _trace:
        return full, res
    return full


# revision 37
# speedup vs baseline: 1.0158x; 1.0089x over previous
"""BitLinear (int4-fakequant x @ ternary-weight linear) Trainium2 Bass kernel.

Math (per reference):
  maxabs[s] = max(|x[s, :]|) clamped to >= 1e-6
  q[s, k]   = round(x[s, k] / maxabs[s] * 7)           # in [-7, 7]
  xq        = q * maxabs / 7
  thresh    = 0.05 * mean(|w|)                          # global scalar
  sign[o,k] = 0 if |w[o,k]| < thresh else sign(w[o,k])  # in {-1, 0, 1}
  alpha[o]  = mean(|w[o, :]|)
  out[s, o] = (maxabs[s]/7) * alpha[o] * S[s,o] + bias[o],  S = q @ sign.T

S is an exact small-integer matmul computed on the PE array in fp8 e4m3 with
DoubleRow (ints -8..7 exact; fp32 accumulation, |S| <= 28672 < 2^24 -> exact).
Per-partition row scale (maxabs/7) applied on ACT during PSUM eviction;
per-column alpha applied on DVE in bf16 (2x mode); output shipped bf16.

Sharding: column-parallel over out_f across 8 cores (weight/alpha/out
sharded, x replicated). Host does layout/stat prep (transposes, row stats,
pre-scale x by 7/maxabs, ternarize w -> fp8 sign), mirroring the baseline's
host-side stats; all O(N*K*O) matmul work plus the quantization rounding
nonlinearity runs on device.

HW-measured facts this schedule is built on (from the baseline trace):
  - a DoubleRow fp8 matmul with N output columns takes ~N/2.4GHz + 3ns; the
    fp8-DR wall is 1 out-col/cycle at 2.4 GHz (157 TF/s).
  - LDWEIGHTS (135 ns) fully hides under the 216 ns matmuls.
  => per-core floor = 64 blocks * 64 MMs * ~216 ns ~= 884 us. The previous
     kernel spent ~1113 us: ~65 us weight-prep serial phase at start plus
     ~150 us of PE gaps from DVE (inv7 multiply) and DMA contention.
"""

import numpy as np
import ml_dtypes

import concourse.bacc as bacc
import concourse.bass as bass
import concourse.mybir as mybir
import concourse.tile as tile
from concourse.bass import ts

F32 = mybir.dt.float32
BF16 = mybir.dt.bfloat16
FP8 = mybir.dt.float8e4
AOP = mybir.AluOpType
ACTF = mybir.ActivationFunctionType

P = 128
OTILE = 512          # psum free-dim tile (one bank of fp32)
MCHUNK = 256         # m-columns per x DMA/quant chunk (2 MM-blocks)
# adding/subtracting this forces RNE round-to-integer in fp32
MAGIC = 1.5 * 2.0 ** 23


def build_nc(M, IN_F, O_SH, with_bias):
    """Per-core SPMD program. Shapes are per-core shard shapes."""
    KSUB = IN_F // P          # k-subtiles (pairs for DoubleRow)
    NPAIR = KSUB // 2
    NCH = M // MCHUNK         # x chunks
    BPC = MCHUNK // P         # MM-blocks per chunk
    NBLK = M // P
    NOT = O_SH // OTILE
    assert KSUB % 2 == 0 and M % MCHUNK == 0 and O_SH % OTILE == 0

    nc = bacc.Bacc("TRN2", target_bir_lowering=False, debug=False)

    # All bulk inputs are packed per-partition-contiguous on the host so each
    # DMA issues one 16-32KB descriptor per partition (512B-line layouts
    # measured only ~130 GB/s).
    # xs: pre-scaled x^T chunks, [NCH, P, KSUB, MCHUNK]
    xs = nc.dram_tensor("xs", [NCH, P, KSUB, MCHUNK], F32, kind="ExternalInput").ap()
    # xs01: chunks 0-1 duplicated as 128-m halves for the fast start
    xs01 = nc.dram_tensor("xs01", [2, BPC, P, KSUB, P], F32, kind="ExternalInput").ap()
    # sg: ternary sign o-quarters, [NOT, P, KSUB, OTILE] fp8
    sg = nc.dram_tensor("sg", [NOT, P, KSUB, OTILE], FP8, kind="ExternalInput").ap()
    rs = nc.dram_tensor("rs", [P, NBLK], F32, kind="ExternalInput").ap()
    alpha = nc.dram_tensor("alpha", [1, O_SH], BF16, kind="ExternalInput").ap()
    if with_bias:
        bias = nc.dram_tensor("bias", [1, O_SH], F32, kind="ExternalInput").ap()
    out = nc.dram_tensor("out", [M, O_SH], BF16, kind="ExternalOutput").ap()

    out_r = out.rearrange("(t p) o -> p t o", p=P)       # [128, NBLK, O_SH]

    with tile.TileContext(nc) as tc:
        with (
            tc.tile_pool(name="const", bufs=1) as constp,
            tc.tile_pool(name="sign", bufs=1) as signp,
            tc.tile_pool(name="xin", bufs=2) as xin,
            tc.tile_pool(name="mini", bufs=1) as minip,
            tc.tile_pool(name="q8p", bufs=3) as q8p,
            tc.tile_pool(name="outp", bufs=2) as outp,
            tc.tile_pool(name="psum", bufs=8, space="PSUM") as psum,
        ):
            # ---- constants (tiny, ahead of sign on the ACT DMA ring) ----
            rs_sb = constp.tile([P, NBLK], F32, tag="rs_sb")
            nc.scalar.dma_start(rs_sb[:], rs[:, :])
            alpha_bc = constp.tile([P, O_SH], BF16, tag="alpha_bc")
            nc.scalar.dma_start(alpha_bc[:], alpha[0:1, :].to_broadcast((P, O_SH)))
            if with_bias:
                bias_bc = constp.tile([P, O_SH], F32, tag="bias_bc")
                nc.scalar.dma_start(bias_bc[:], bias[0:1, :].to_broadcast((P, O_SH)))

            # ---- sign tiles: straight fp8 DMA, one per o-quarter; quarter q
            # is first consumed ~7*q us into the run ----
            # two k-half tiles per o-quarter (per-tile dep tracking: the
            # first 8 matmul pairs need only 1.05 MB of sign), loads split
            # across both DMA rings so arrival tracks consumption:
            # q0/q1 halves on the sync ring interleaved with the first x
            # minis, q2/q3 on the scalar ring behind rs/alpha
            KH = KSUB // 2
            sign_ts = [
                [
                    signp.tile(
                        [P, KH, OTILE], FP8, tag=f"sign{q}{h}", name=f"sign{q}{h}"
                    )
                    for h in range(2)
                ]
                for q in range(NOT)
            ]
            nc.sync.dma_start(sign_ts[0][0][:], sg[0][:, 0:KH, :])
            for q in range(2, NOT):
                for h in range(2):
                    nc.scalar.dma_start(
                        sign_ts[q][h][:], sg[q][:, h * KH : (h + 1) * KH, :]
                    )

            def quant_chunk(c):
                xt_t = xin.tile([P, KSUB, MCHUNK], F32, tag="xt", name=f"xt_{c}")
                q8_t = q8p.tile([P, KSUB, MCHUNK], FP8, tag="q8", name=f"q8_{c}")
                nc.sync.dma_start(xt_t[:], xs[c])
                nc.vector.tensor_scalar(
                    q8_t[:], xt_t[:], MAGIC, -MAGIC, AOP.add, AOP.add
                )
                return q8_t

            def quant_minis(c, sign_after=None):
                # chunk as two independent 128-m tiles: block (c,0) unblocks
                # after 2 MB of x instead of 4 (tile-granular dep tracking)
                outs = []
                for h in range(BPC):
                    xt_t = minip.tile(
                        [P, KSUB, P], F32, tag=f"xtm{h}", name=f"xtm_{c}_{h}"
                    )
                    nc.sync.dma_start(xt_t[:], xs01[c, h])
                    if sign_after is not None:
                        # q0's second half / q1's halves ride the sync ring
                        # between the x minis
                        if h == 0:
                            nc.sync.dma_start(
                                sign_ts[0][1][:], sg[0][:, KH:, :]
                            )
                            nc.sync.dma_start(
                                sign_ts[sign_after][0][:],
                                sg[sign_after][:, 0:KH, :],
                            )
                        else:
                            nc.sync.dma_start(
                                sign_ts[sign_after][1][:],
                                sg[sign_after][:, KH:, :],
                            )
                    q8_t = minip.tile(
                        [P, KSUB, P], FP8, tag=f"q8m{h}", name=f"q8m_{c}_{h}"
                    )
                    nc.vector.tensor_scalar(
                        q8_t[:], xt_t[:], MAGIC, -MAGIC, AOP.add, AOP.add
                    )
                    outs.append(q8_t)
                return outs

            # ---- main pipeline: supergroup == one m-chunk (2 blocks); the
            # o-quarter loop is OUTSIDE the block loop so quarter q of sign
            # is needed only q*2*3.45us into each supergroup — the PE rides
            # the sign DMA arrival curve instead of waiting for all of it
            q8_next = [None] * (NCH + 1)
            q8_next[0] = quant_minis(0, sign_after=1)
            if NCH > 1:
                q8_next[1] = quant_minis(1)
            for c in range(NCH):
                q8_t = q8_next[c]
                minis = isinstance(q8_t, list)
                if c + 2 < NCH:
                    q8_next[c + 2] = quant_chunk(c + 2)
                obs = [
                    outp.tile([P, O_SH], BF16, tag="ob", name=f"ob_{c}_{b}")
                    for b in range(BPC)
                ]
                for q in range(NOT):
                    for b in range(BPC):
                        t = c * BPC + b      # global m-block id
                        lq = q8_t[b] if minis else q8_t
                        msl = ts(0 if minis else b, P)
                        ps_t = psum.tile(
                            [P, OTILE], F32, tag="ps", name=f"ps_{t}_{q}"
                        )
                        for kk in range(NPAIR):
                            kh, kl = divmod(2 * kk, KH)
                            nc.tensor.matmul(
                                ps_t[:],
                                lq[:, 2 * kk : 2 * kk + 2, msl],
                                sign_ts[q][kh][:, kl : kl + 2, :],
                                start=(kk == 0),
                                stop=(kk == NPAIR - 1),
                                perf_mode=mybir.MatmulPerfMode.DoubleRow,
                            )
                        nc.scalar.activation(
                            obs[b][:, ts(q, OTILE)],
                            ps_t[:],
                            ACTF.Copy,
                            scale=rs_sb[:, t : t + 1],
                        )
                        # per-quarter alpha + store: keeps the post-matmul
                        # tail to one quarter's chain instead of two blocks'
                        nc.vector.tensor_tensor(
                            obs[b][:, ts(q, OTILE)],
                            obs[b][:, ts(q, OTILE)],
                            alpha_bc[:, ts(q, OTILE)],
                            AOP.mult,
                        )
                        if with_bias:
                            nc.vector.tensor_tensor(
                                obs[b][:, ts(q, OTILE)],
                                obs[b][:, ts(q, OTILE)],
                                bias_bc[:, ts(q, OTILE)],
                                AOP.add,
                            )
                        nc.scalar.dma_start(
                            out_r[:, t, ts(q, OTILE)], obs[b][:, ts(q, OTILE)]
                        )

    nc.compile()
    return nc


def host_prep(x, weight, bias, n_cores):
    """Host-side layout prep + row stats + ternarize. Returns per-core maps."""
    IN_F = x.shape[-1]
    OUT_F = weight.shape[0]
    M = int(np.prod(x.shape[:-1]))
    O_SH = OUT_F // n_cores
    NBLK = M // P
    NCH = M // MCHUNK

    x2 = np.ascontiguousarray(x.reshape(M, IN_F), dtype=np.float32)
    maxabs = np.maximum(np.abs(x2).max(axis=1), 1e-6).astype(np.float32)
    # exact reference order: (x / maxabs) * 7, all fp32
    xs2 = ((x2 / maxabs[:, None]).astype(np.float32) * np.float32(7.0)).astype(
        np.float32
    )
    rs = (maxabs / np.float32(7.0)).astype(np.float32)
    rs_striped = np.ascontiguousarray(rs.reshape(NBLK, P).T)  # [128, NBLK]

    KSUB = IN_F // P
    BPC = MCHUNK // P
    xsT = xs2.T  # [IN_F, M]
    # per-partition-contiguous chunks: [NCH, P, KSUB, MCHUNK]
    xs_cm = np.ascontiguousarray(
        xsT.reshape(KSUB, P, NCH, MCHUNK).transpose(2, 1, 0, 3)
    )
    # chunks 0-1 as 128-m halves: [2, BPC, P, KSUB, P]
    xs01 = np.ascontiguousarray(
        xsT[:, : 2 * MCHUNK]
        .reshape(KSUB, P, 2, BPC, P)
        .transpose(2, 3, 1, 0, 4)
    )

    w64 = weight.astype(np.float32)
    thresh = np.float32(0.05) * np.float32(np.abs(w64).mean(dtype=np.float64))
    sign_full = np.where(np.abs(w64) < thresh, 0.0, np.sign(w64)).astype(
        ml_dtypes.float8_e4m3fn
    )  # [OUT_F, IN_F]
    alpha_full = np.abs(w64).mean(axis=1, dtype=np.float32)
    NOT = O_SH // OTILE

    with_bias = bool(np.any(bias))

    in_maps = []
    for c in range(n_cores):
        o0, o1 = c * O_SH, (c + 1) * O_SH
        sgT = sign_full[o0:o1].T  # [IN_F, O_SH] fp8
        m = {
            "xs": xs_cm,
            "xs01": xs01,
            # o-quartered per-partition-contiguous: [NOT, P, KSUB, OTILE]
            "sg": np.ascontiguousarray(
                sgT.reshape(KSUB, P, NOT, OTILE).transpose(2, 1, 0, 3)
            ),
            "rs": rs_striped,
            "alpha": alpha_full[o0:o1].astype(ml_dtypes.bfloat16).reshape(1, O_SH),
        }
        if with_bias:
            m["bias"] = np.ascontiguousarray(bias[o0:o1], dtype=np.float32).reshape(
                1, O_SH
            )
        in_maps.append(m)
    return in_maps, with_bias


_NC_CACHE = {}


def _get_nc(M, IN_F, O_SH, with_bias):
    key = (M, IN_F, O_SH, with_bias)
    if key not in _NC_CACHE:
        _NC_CACHE[key] = build_nc(M, IN_F, O_SH, with_bias)
    return _NC_CACHE[key]


def kernel(x, weight, bias, _trace=False):
    from concourse.bass_utils import run_bass_kernel_spmd

    N_CORES = 8
    x = np.asarray(x)
    weight = np.asarray(weight)
    bias = np.asarray(bias)
    IN_F = x.shape[-1]
    OUT_F = weight.shape[0]
    M = int(np.prod(x.shape[:-1]))
    O_SH = OUT_F // N_CORES

    in_maps, with_bias = host_prep(x, weight, bias, N_CORES)
    nc = _get_nc(M, IN_F, O_SH, with_bias)
    res = run_bass_kernel_spmd(
        nc, in_maps, core_ids=list(range(N_CORES)), trace=_trace
    )
    parts = [
        res.results[c]["out"].astype(np.float32).reshape(*x.shape[:-1], O_SH)
        for c in range(N_CORES)
    ]
    full = np.concatenate(parts, axis=-1)
    if with_bias is False and np.any(bias):  # pragma: no cover (safety)
        full = full + bias
    if _trace:
        return full, res
    return full
